# revision 2
# baseline (speedup 1.0000x reference)
"""Trainium2 Bass kernel for a token-embedding LSTM:
    x = emb[tokens]                               [B, T, E]
    LSTM over T steps (units=512), final h_T
    out = sigmoid(h_T @ W + b)                    [B, 1]

Sharding: data-parallel over batch. B=256 split as 32 rows per core
across 8 NeuronCores; weights replicated; no collectives.

Per-core dataflow:
  Phase B: gather embedding rows (time-major order), PE-transpose them,
    and precompute xproj = x @ [Wf|Wi|Wc|Wo] + b for all T*32 rows into
    DRAM (bf16). The bias is folded in with a K=1 ones-row matmul so the
    PSUM->SBUF move can run on the scalar engine.
  Phase C: sequential LSTM over T steps. Gate matmuls keep the
    (transposed) hidden state as the stationary operand ([128,32] tiles,
    cheap weight loads) and stream the recurrent weights U in bf16; the
    precomputed xproj is injected into the PSUM accumulation with an
    identity matmul. The per-step hidden-state transpose is one 128x128
    PE transpose of a strip-stacked h tile.
"""

import os
import sys

import numpy as np
import ml_dtypes

sys.path.insert(0, "/opt/trn_rl_repo")

import concourse.bacc as bacc
import concourse.bass as bass
import concourse.mybir as mybir
import concourse.tile as tile
from concourse.bass_utils import run_bass_kernel_spmd

F32 = mybir.dt.float32
BF16 = mybir.dt.bfloat16
I32 = mybir.dt.int32
AF = mybir.ActivationFunctionType

N_CORES = 8
B = 256
B_LOC = B // N_CORES  # 32
T_FULL = 512
EMB = 256
UNITS = 512
G = 4 * UNITS  # 2048 concatenated gate width, order [f | i | c | o]
VOCAB = 50000


def build_nc(T=T_FULL, unroll=32, num_devices=N_CORES):
    """Build the per-core Bass program. Same program runs on all cores."""
    rows = T * B_LOC
    n_mtiles = rows // 128
    assert rows % 128 == 0
    assert T % unroll == 0 and unroll % 4 == 0
    u2 = unroll // 4

    nc = bacc.Bacc(
        "TRN2", target_bir_lowering=False, debug=False, num_devices=num_devices
    )

    tokens_pm = nc.dram_tensor(
        "tokens_pm", [128, n_mtiles], I32, kind="ExternalInput"
    ).ap()
    emb_d = nc.dram_tensor("emb", [VOCAB, EMB], BF16, kind="ExternalInput").ap()
    wcat_d = nc.dram_tensor("wcat", [EMB, G], BF16, kind="ExternalInput").ap()
    ucat_d = nc.dram_tensor("ucat", [UNITS, G], BF16, kind="ExternalInput").ap()
    ones_d = nc.dram_tensor("ones", [1, 128], BF16, kind="ExternalInput").ap()
    bb_d = nc.dram_tensor("bb", [128, 1024], F32, kind="ExternalInput").ap()
    brow_d = nc.dram_tensor("brow", [1, G], BF16, kind="ExternalInput").ap()
    ident_d = nc.dram_tensor("ident", [128, 128], BF16, kind="ExternalInput").ap()
    wout_d = nc.dram_tensor("wout", [128, 4], BF16, kind="ExternalInput").ap()
    bout_d = nc.dram_tensor("bout", [B_LOC, 1], F32, kind="ExternalInput").ap()
    y_d = nc.dram_tensor("y", [B_LOC, 1], F32, kind="ExternalOutput").ap()

    with tile.TileContext(nc) as tc:
        with (
            tc.tile_pool(name="const", bufs=1) as constp,
            tc.tile_pool(name="dram", bufs=1, space="DRAM") as dramp,
        ):
            # resident constants
            u_sb = []
            for k in range(4):
                t = constp.tile([128, G], BF16, tag=f"u{k}")
                nc.sync.dma_start(t[:], ucat_d[k * 128 : (k + 1) * 128, :])
                u_sb.append(t)
            w_sb = []
            for c in range(2):
                t = constp.tile([128, G], BF16, tag=f"w{c}")
                nc.sync.dma_start(t[:], wcat_d[c * 128 : (c + 1) * 128, :])
                w_sb.append(t)
            ones_sb = constp.tile([1, 128], BF16, tag="ones")
            nc.sync.dma_start(ones_sb[:], ones_d[:])
            bb_sb = constp.tile([128, 1024], F32, tag="bb")
            nc.sync.dma_start(bb_sb[:], bb_d[:])
            brow_sb = constp.tile([1, G], BF16, tag="brow")
            nc.sync.dma_start(brow_sb[:], brow_d[:])
            id_sb = constp.tile([128, 128], BF16, tag="ident")
            nc.sync.dma_start(id_sb[:], ident_d[:])
            wout_sb = constp.tile([128, 4], BF16, tag="wout")
            nc.sync.dma_start(wout_sb[:], wout_d[:])
            bout_sb = constp.tile([B_LOC, 1], F32, tag="bout")
            nc.sync.dma_start(bout_sb[:], bout_d[:])
            tok_sb = constp.tile([128, n_mtiles], I32, tag="tok")
            nc.sync.dma_start(tok_sb[:], tokens_pm[:])

            xproj = dramp.tile([rows, G], BF16)

            # ---- Phase B: gather + transpose + xproj precompute ----
            with (
                tc.tile_pool(name="gat", bufs=3) as gatp,
                tc.tile_pool(name="xtp", bufs=3) as xtp,
                tc.tile_pool(name="xpo", bufs=3) as xpop,
                tc.tile_pool(name="psB", bufs=2, space="PSUM") as psB,
                tc.tile_pool(name="psX", bufs=1, space="PSUM") as psX,
            ):
                for m in range(n_mtiles):
                    xg = gatp.tile([128, EMB], BF16, tag="xg")
                    nc.gpsimd.indirect_dma_start(
                        out=xg[:],
                        out_offset=None,
                        in_=emb_d[:],
                        in_offset=bass.IndirectOffsetOnAxis(
                            ap=tok_sb[:, m : m + 1], axis=0
                        ),
                    )
                    xts = []
                    for c in range(2):
                        trp = psB.tile([128, 128], BF16, tag="trp")
                        nc.tensor.transpose(
                            trp[:], xg[:, c * 128 : (c + 1) * 128], id_sb[:]
                        )
                        xt = xtp.tile([128, 128], BF16, tag="xt")
                        nc.vector.tensor_copy(xt[:], trp[:])
                        xts.append(xt)
                    xpo = xpop.tile([128, G], BF16, tag="xpo")
                    for j in range(4):
                        nsl = slice(j * 512, (j + 1) * 512)
                        xps = psX.tile([128, 512], F32, tag=f"xps{j}")
                        first = True
                        if j >= 2:
                            nc.tensor.matmul(
                                xps[:],
                                lhsT=ones_sb[:, :],
                                rhs=brow_sb[:, nsl],
                                start=True,
                                stop=False,
                            )
                            first = False
                        for c in range(2):
                            nc.tensor.matmul(
                                xps[:],
                                lhsT=xts[c][:],
                                rhs=w_sb[c][:, nsl],
                                start=first,
                                stop=(c == 1),
                            )
                            first = False
                        if j < 2:
                            nc.vector.tensor_add(
                                xpo[:, nsl], xps[:], bb_sb[:, nsl]
                            )
                        else:
                            nc.scalar.copy(xpo[:, nsl], xps[:])
                    nc.gpsimd.dma_start(xproj[m * 128 : (m + 1) * 128, :], xpo[:])

            # ---- Phase C: recurrence ----
            with (
                tc.tile_pool(name="state", bufs=1) as statep,
                tc.tile_pool(name="xin", bufs=3) as xinp,
                tc.tile_pool(name="gsb", bufs=2) as gsbp,
                tc.tile_pool(name="tmp", bufs=2) as tmpp,
                tc.tile_pool(name="hsp", bufs=2) as hsp,
                tc.tile_pool(name="psG", bufs=1, space="PSUM") as psG,
                tc.tile_pool(name="psH", bufs=1, space="PSUM") as psH,
            ):
                hT_sb = statep.tile([128, 128], BF16, tag="hT")
                c_sb = statep.tile([B_LOC, UNITS], F32, tag="c")
                nc.vector.memset(hT_sb[:], 0.0)
                nc.vector.memset(c_sb[:], 0.0)

                def halfload(row0):
                    """DMA u2 steps of xproj ([u2*32, G] rows) into one
                    [32, u2*G] tile (batch in partitions, steps along free)."""
                    xq = xinp.tile([B_LOC, u2 * G], BF16, tag="xq")
                    src = xproj[row0, :].rearrange("(s b) g -> b s g", b=B_LOC)
                    nc.sync.dma_start(xq[:].rearrange("b (s g) -> b s g", s=u2), src)
                    return xq

                def mm_inject(out_ap, xsl_lo, xsl_hi, xq):
                    """Start a gate-bank accumulation with its xproj slice.
                    Emitted as a standalone group-opener so the scheduler can
                    run it while the previous step's h-tail is still in
                    flight (it has no dependency on the new hidden state)."""
                    nc.tensor.matmul(
                        out_ap,
                        lhsT=id_sb[0:B_LOC, 0:B_LOC],
                        rhs=xq[:, xsl_lo:xsl_hi],
                        start=True,
                        stop=False,
                        skip_group_check=True,
                    )

                def mm_group(out_ap, usl):
                    """4 U-chunk matmuls accumulating onto the injected bank."""
                    for k in range(4):
                        nc.tensor.matmul(
                            out_ap,
                            lhsT=hT_sb[:, k * 32 : (k + 1) * 32],
                            rhs=u_sb[k][:, usl],
                            start=False,
                            stop=(k == 3),
                            skip_group_check=True,
                        )

                def step(xq, s_local):
                    """One LSTM step using xq slice s_local."""
                    x0 = s_local * G
                    gsb = gsbp.tile([B_LOC, G], F32, tag="gsb")
                    # open all six gate-bank accumulations with their xproj
                    # slices first (independent of the recurrent state)
                    gf = psG.tile([B_LOC, 512], F32, tag="gpsf")
                    gi = psG.tile([B_LOC, 512], F32, tag="gpsi")
                    gc0 = psG.tile([B_LOC, 256], F32, tag="gpsc0")
                    gc1 = psG.tile([B_LOC, 256], F32, tag="gpsc1")
                    go0 = psG.tile([B_LOC, 256], F32, tag="gpso0")
                    go1 = psG.tile([B_LOC, 256], F32, tag="gpso1")
                    gc = [gc0, gc1]
                    go = [go0, go1]
                    mm_inject(gf[:], x0, x0 + 512, xq)
                    mm_inject(gi[:], x0 + 512, x0 + 1024, xq)
                    for hh in range(2):
                        lo = 1024 + hh * 256
                        mm_inject(gc[hh][:], x0 + lo, x0 + lo + 256, xq)
                    for hh in range(2):
                        lo = 1536 + hh * 256
                        mm_inject(go[hh][:], x0 + lo, x0 + lo + 256, xq)
                    # f, i banks
                    mm_group(gf[:], slice(0, 512))
                    nc.scalar.activation(gsb[:, 0:512], gf[:], AF.Sigmoid)
                    ct1 = tmpp.tile([B_LOC, UNITS], F32, tag="ct1")
                    nc.vector.tensor_mul(ct1[:], gsb[:, 0:512], c_sb[:])
                    mm_group(gi[:], slice(512, 1024))
                    nc.scalar.activation(gsb[:, 512:1024], gi[:], AF.Sigmoid)
                    # chat in two half-banks so the c-chain starts earlier
                    ct2 = tmpp.tile([B_LOC, UNITS], F32, tag="ct2")
                    thc = tmpp.tile([B_LOC, UNITS], F32, tag="thc")
                    for hh in range(2):
                        lo = 1024 + hh * 256
                        mm_group(gc[hh][:], slice(lo, lo + 256))
                        csl = slice(lo, lo + 256)
                        usl = slice(hh * 256, (hh + 1) * 256)
                        nc.scalar.activation(gsb[:, csl], gc[hh][:], AF.Tanh)
                        nc.vector.tensor_mul(
                            ct2[:, usl], gsb[:, 512 + hh * 256 : 512 + (hh + 1) * 256],
                            gsb[:, csl],
                        )
                        nc.vector.tensor_add(c_sb[:, usl], ct1[:, usl], ct2[:, usl])
                        nc.scalar.activation(thc[:, usl], c_sb[:, usl], AF.Tanh)
                    # o gate in two 256-col half-banks pipelined through
                    # sigmoid and the h-strip multiplies
                    hs = hsp.tile([128, 128], BF16, tag="hs")
                    for hh in range(2):
                        lo = 1536 + hh * 256
                        mm_group(go[hh][:], slice(lo, lo + 256))
                        osl = slice(lo, lo + 256)
                        nc.scalar.activation(gsb[:, osl], go[hh][:], AF.Sigmoid)
                        for c in (2 * hh, 2 * hh + 1):
                            nc.vector.tensor_mul(
                                hs[c * 32 : (c + 1) * 32, :],
                                gsb[:, 1536 + c * 128 : 1536 + (c + 1) * 128],
                                thc[:, c * 128 : (c + 1) * 128],
                            )
                    htp = psH.tile([128, 128], BF16, tag="htp")
                    nc.tensor.transpose(htp[:], hs[:], id_sb[:])
                    nc.vector.tensor_copy(hT_sb[:], htp[:])

                def iteration(iv):
                    for q in range(4):
                        xq = halfload(bass.ds(iv + q * (u2 * B_LOC), u2 * B_LOC))
                        for s in range(u2):
                            step(xq, s)

                n_iters = T // unroll
                if n_iters == 1:
                    iteration(0)
                else:
                    with tc.For_i(
                        0,
                        rows,
                        B_LOC * unroll,
                        staggered_reset=True,
                        hint_engines=(
                            mybir.EngineType.PE,
                            mybir.EngineType.DVE,
                            mybir.EngineType.Activation,
                        ),
                    ) as iv:
                        iteration(iv)

                # final projection + sigmoid
                yps = psH.tile([B_LOC, 1], F32, tag="htp")
                for k in range(4):
                    nc.tensor.matmul(
                        yps[:],
                        lhsT=hT_sb[:, k * 32 : (k + 1) * 32],
                        rhs=wout_sb[:, k : k + 1],
                        start=(k == 0),
                        stop=(k == 3),
                    )
                ysb = tmpp.tile([B_LOC, 1], F32, tag="ysb")
                nc.scalar.activation(ysb[:], yps[:], AF.Sigmoid, bias=bout_sb[:, 0:1])
                nc.sync.dma_start(y_d[:], ysb[:])

    nc.compile()
    return nc


def prep_inputs(tokens, emb, Wf, Uf, bf, Wi, Ui, bi, Wc, Uc, bc, Wo, Uo, bo, W, b):
    """Host-side prep: concat gate weights, cast to bf16, shard tokens."""
    bf16 = ml_dtypes.bfloat16
    wcat = np.concatenate([Wf, Wi, Wc, Wo], axis=1).astype(bf16)  # [E, G]
    ucat = np.concatenate([Uf, Ui, Uc, Uo], axis=1).astype(bf16)  # [U, G]
    bcat = np.concatenate([bf, bi, bc, bo], axis=0).astype(np.float32)  # [G]
    brow = bcat[None, :].astype(bf16)
    bb = np.broadcast_to(bcat[None, :1024], (128, 1024)).copy()
    ones = np.ones((1, 128), bf16)
    emb_bf = np.asarray(emb, np.float32).astype(bf16)
    ident = np.eye(128, dtype=bf16)
    wout = np.ascontiguousarray(
        np.asarray(W, np.float32).reshape(4, 128).T
    ).astype(bf16)  # [128, 4]; wout[p, k] = W[k*128 + p]
    bout = np.full((B_LOC, 1), float(np.asarray(b).reshape(-1)[0]), np.float32)

    tokens = np.asarray(tokens)
    T = tokens.shape[1]
    n_mtiles = T * B_LOC // 128
    per_core = []
    for core in range(N_CORES):
        tok = tokens[core * B_LOC : (core + 1) * B_LOC]  # [B_LOC, T]
        tok_tm = np.ascontiguousarray(tok.T).reshape(-1)  # row = t*B_LOC + b
        tok_pm = np.ascontiguousarray(
            tok_tm.reshape(n_mtiles, 128).T
        ).astype(np.int32)  # [128, n_mtiles]
        per_core.append(
            dict(
                tokens_pm=tok_pm,
                emb=emb_bf,
                wcat=wcat,
                ucat=ucat,
                ones=ones,
                brow=brow,
                bb=bb,
                ident=ident,
                wout=wout,
                bout=bout,
            )
        )
    return per_core


_NC_CACHE = {}
LAST_RESULT = None


def kernel(**inputs):
    global LAST_RESULT
    key = "full"
    if key not in _NC_CACHE:
        _NC_CACHE[key] = build_nc()
    nc = _NC_CACHE[key]
    in_maps = prep_inputs(**inputs)
    res = run_bass_kernel_spmd(nc, in_maps, core_ids=list(range(N_CORES)))
    LAST_RESULT = res
    out = np.concatenate([r["y"] for r in res.results], axis=0)
    return out.astype(np.float32)

